# revision 36
# baseline (speedup 1.0000x reference)
"""MetaPathGNN kernel for 8 Trainium2 NeuronCores.

Computation (h_b/conv0/edge_ab/x_b are dead code in the reference):
    msg  = x_a[edge_ba[1]]                      # [E, H] gather
    aggr = segment_sum(msg, edge_ba[0], N)      # [N, H]
    h_a  = relu(aggr @ wl1.T + x_a @ (w01+w11).T + (bl1+b01+b11))
    out  = h_a @ out_w.T + out_b

Sharding: destination nodes are dealt to the 8 cores round-robin in
descending-degree order (so the shared SPMD chunk budgets, which are maxed
across cores, see near-identical per-window loads), stratified across PSUM
groups so group weights are uniform. Each core gathers source rows for its
own edges from a full x_a replica (no collectives), aggregates via one-hot
matmuls into PSUM, applies the linears feature-major, writes its outT
stripe; the host un-permutes and reassembles.

Cost-model structure this kernel is built around (measured):
  - SWDGE gather transfers serialize at ~0.833 ns/row (256B f16 rows) and are
    the wall; everything else must hide under them.
  - HWDGE (regular dma_start) traffic runs fully in parallel with the SWDGE
    stream (~330 GB/s), so the one-hot S matrices are PRECOMPUTED ON HOST,
    stored fp8 (exact for 0/1), and streamed from HBM instead of being built
    on DVE (matmul accepts mixed f16 lhsT x fp8 rhs).
  - Each dma_start costs ~0.5-0.8us of SP engine time regardless of size, so
    loads are batched coarsely (whole idx stream, whole xaT, few S batches).

Aggregation: per 250-dest PSUM group (25 groups), windows of <=128 dests are
chosen by a small DP to minimize the shared chunk budget sum_w max_core
ceil(rows/128) on source-DEDUPED row counts (a gathered row's S row is
multi-hot over all its edges in the window; duplicate edges add 2). Pad
chunks gather row 0 with an all-zero S block. Window matmuls out = msg.T @ S
accumulate bank-wide (start=True pending-zeroes the bank; every window is
touched by >=1 matmul). The last TAILG groups get group-aligned gather
batches so their post chains pipeline with the stream instead of piling up
after it.

dma_gather indices are int16, so sources are gathered in two passes with
OVERLAPPING bases (pass A: rows [0, 32768), pass B: rows [17232, 50000)) --
edges with src in the overlap can ride in either pass, which merges the
per-window per-pass ceil waste in the shared chunk budgets.
"""

import numpy as np

P = 8
N = 50000
E = 500000
H = 128
NSH = N // P          # 6250 destinations per core
GROUP = 256           # aggregation group width (<= 512 psum bank cols)
NG = (NSH + GROUP - 1) // GROUP   # 25
NRG = NSH // NG       # 250 real dests per group (stratified deal)
NCOL = NG * GROUP
SPLIT = 32768         # int16-index limit for dma_gather
OVL = N - SPLIT       # 17232: pass-B gather base; src in [OVL, SPLIT) fits either pass
WMAX = 128            # max window width
GCAP = 48             # gather batch cap, chunks
SCAP = 4096           # S-stream batch cap, columns


def _bucket_budget(cc, s, e):
    """Shared (bA, bB) chunk budget for window [s, e).

    cc: [P, 3, n+1] cumsums by class (0=must-be-A src<OVL, 1=flexible,
    2=must-be-B src>=SPLIT). Flexible edges may go in either pass, so the
    total budget is max(ceil(total), ceil(minA)+ceil(minB)) per core-max.
    """
    nAm = int((cc[:, 0, e] - cc[:, 0, s]).max())
    nBm = int((cc[:, 2, e] - cc[:, 2, s]).max())
    nT = int(((cc[:, 0, e] + cc[:, 1, e] + cc[:, 2, e])
              - (cc[:, 0, s] + cc[:, 1, s] + cc[:, 2, s])).max())
    LA = -(-nAm // 128)
    LB = -(-nBm // 128)
    bt = max(-(-nT // 128), LA + LB, 1)
    bA = max(LA, bt - LB)
    return bA, bt - bA


def _dp_windows(cc, nreal):
    """Choose window boundaries for one group: tile [0, nreal) with windows
    <= WMAX wide, minimizing total chunk budget (ties: fewer matmul cols)."""
    INF = 1 << 40
    dp = np.full(nreal + 1, INF, np.int64)
    dp[0] = 0
    choice = np.zeros(nreal + 1, np.int64)
    ccT = cc.sum(axis=1)
    for e in range(1, nreal + 1):
        w = np.arange(1, min(WMAX, e) + 1)
        s = e - w
        nAm = (cc[:, 0, e:e + 1] - cc[:, 0, s]).max(axis=0)
        nBm = (cc[:, 2, e:e + 1] - cc[:, 2, s]).max(axis=0)
        nT = (ccT[:, e:e + 1] - ccT[:, s]).max(axis=0)
        bt = np.maximum(np.maximum(-(-nT // 128), (-(-nAm // 128)) + (-(-nBm // 128))), 1)
        cost = dp[s] + bt * 100000 + bt * w
        i = int(np.argmin(cost))
        dp[e] = cost[i]
        choice[e] = w[i]
    wins = []
    e = nreal
    while e > 0:
        w = int(choice[e])
        s = e - w
        bA, bB = _bucket_budget(cc, s, e)
        wins.append((s, w, bA, bB))
        e = s
    return wins[::-1]


def _pack_edges(dst, src, core_of, dl_of):
    """Window selection, shared slot schedule, and per-core stream packing.

    Returns (slots, CA, CB, SCOLS, per_core).
    slots: list of dicts with p, g, woff, width, scol, chunk (per-pass chunk
    index), first/last (of PSUM group) in processing order.
    per_core[c]: idxA/idxB int16 streams and S fp8 [128, SCOLS].
    """
    core = core_of[dst]
    dl = dl_of[dst]
    g_of = dl // GROUP
    lo = dl - g_of * GROUP
    cls = np.where(src < OVL, 0, np.where(src < SPLIT, 1, 2)).astype(np.int64)

    # per-group per-core per-class cumulative counts -> DP windows
    group_wins = []
    for g in range(NG):
        nreal = NRG
        cc = np.zeros((P, 3, nreal + 1), np.int64)
        m = g_of == g
        for c in range(P):
            for k in range(3):
                mm = m & (core == c) & (cls == k)
                cnt = np.bincount(lo[mm], minlength=nreal)
                cc[c, k, 1:] = np.cumsum(cnt)
        group_wins.append(_dp_windows(cc, nreal))

    # process heavy groups first so the tail chain is light
    gweight = [sum(bA + bB for _, _, bA, bB in group_wins[g]) for g in range(NG)]
    group_order = sorted(range(NG), key=lambda g: -gweight[g])

    slots = []
    ca = cb = scols = 0
    for g in group_order:
        first = len(slots)
        for woff, w, bA, bB in group_wins[g]:
            for p, b in ((0, bA), (1, bB)):
                for _ in range(b):
                    slots.append(dict(p=p, g=g, woff=woff, width=w, scol=scols,
                                      chunk=(ca if p == 0 else cb),
                                      first=False, last=False))
                    scols += w
                    if p == 0:
                        ca += 1
                    else:
                        cb += 1
        slots[first]["first"] = True
        slots[-1]["last"] = True
    CA, CB, SCOLS = ca, cb, scols

    # per-core packing: bucket edges by (g, window, pass), chunk, emit streams
    import ml_dtypes
    win_id = np.zeros(len(dl), np.int64)
    win_off = np.zeros(len(dl), np.int64)
    wid_base = {}
    wb = 0
    for g in range(NG):
        bounds = np.array([woff for woff, _, _, _ in group_wins[g]] + [GROUP])  # edges all < nreal
        m = g_of == g
        wi = np.searchsorted(bounds, lo[m], side="right") - 1
        win_id[m] = wb + wi
        win_off[m] = lo[m] - bounds[wi]
        wid_base[g] = wb
        wb += len(group_wins[g])

    # ---- row-level (source-deduped) packing ----
    # A gathered row is (window, source); its S row is multi-hot over the
    # window columns of all its edges (duplicate (src,dst) edges add 2).
    # Budgets are therefore computed on DISTINCT-source counts per window.
    NWID = wb
    row_data = []   # per core: (row_win, row_src, row_cls, edge_inv, edge_scolpos)
    cnts = np.zeros((P, NWID, 3), np.int64)
    for c in range(P):
        m = core == c
        w_ = win_id[m]
        s_ = src[m]
        pair = w_ * N + s_
        uniq, inv = np.unique(pair, return_inverse=True)
        r_win = uniq // N
        r_src = uniq % N
        r_cls = np.where(r_src < OVL, 0, np.where(r_src < SPLIT, 1, 2))
        np.add.at(cnts[c], (r_win, r_cls), 1)
        row_data.append((r_win, r_src, r_cls, inv, win_off[m]))

    # shared budgets per window from distinct counts (flex merges the ceils)
    def _budget(wid):
        nAm = int(cnts[:, wid, 0].max())
        nBm = int(cnts[:, wid, 2].max())
        nT = int(cnts[:, wid, :].sum(axis=1).max())
        LA = -(-nAm // 128)
        LB = -(-nBm // 128)
        bt = max(-(-nT // 128), LA + LB, 1)
        bA = max(LA, bt - LB)
        return bA, bt - bA

    new_wins = []
    for g in range(NG):
        nw = []
        for i, (woff, w, _, _) in enumerate(group_wins[g]):
            bA, bB = _budget(wid_base[g] + i)
            nw.append((woff, w, bA, bB))
        new_wins.append(nw)
    group_wins = new_wins

    # rebuild slots with the tightened budgets
    gweight = [sum(bA + bB for _, _, bA, bB in group_wins[g]) for g in range(NG)]
    group_order = sorted(range(NG), key=lambda g: -gweight[g])
    slots = []
    ca = cb = scols = 0
    pg = 0
    for gi, g in enumerate(group_order):
        wins = group_wins[g]
        if gi >= len(group_order) - 2 and len(wins) > 1:
            # final group: two pseudo-groups (separate PSUM banks + stop
            # flags + batches) so its first half-chain starts ~1us before
            # the gather stream ends
            k = min(range(1, len(wins)),
                    key=lambda i: abs(wins[i][0] - NRG // 2))
            subs = [(wins[:k], (0, wins[k][0])), (wins[k:], (wins[k][0], NRG))]
        else:
            subs = [(wins, (0, NRG))]
        for sub, rng in subs:
            first = len(slots)
            for woff, w, bA, bB in sub:
                for p, b in ((0, bA), (1, bB)):
                    for _ in range(b):
                        slots.append(dict(p=p, g=g, pg=pg, woff=woff, width=w,
                                          scol=scols,
                                          chunk=(ca if p == 0 else cb),
                                          first=False, last=False, rng=rng))
                        scols += w
                        if p == 0:
                            ca += 1
                        else:
                            cb += 1
            slots[first]["first"] = True
            slots[-1]["last"] = True
            pg += 1
    CA, CB, SCOLS = ca, cb, scols

    budA = {}
    for g in range(NG):
        for i, (woff, w, bA, bB) in enumerate(group_wins[g]):
            budA[wid_base[g] + i] = bA

    per_core = []
    for c in range(P):
        r_win, r_src, r_cls, inv, e_off = row_data[c]
        nrows = len(r_win)
        # rows ordered by (window, class, src); rank within window
        order = np.lexsort((r_src, r_cls, r_win))
        rank = np.empty(nrows, np.int64)
        rank[order] = np.arange(nrows)
        uw, w_first = np.unique(r_win[order], return_index=True)
        wstart = {int(u): int(fi) for u, fi in zip(uw, w_first)}
        wcount = {int(u): int(n) for u, n in
                  zip(uw, np.bincount(np.searchsorted(uw, r_win[order])))}
        # A-row count per window: must-A + flex up to the A budget
        nA = {}
        for u in uw:
            u = int(u)
            n_mustA = int(cnts[c, u, 0])
            n_flex = int(cnts[c, u, 1])
            nA[u] = min(n_mustA + n_flex, 128 * budA[u])
        # map each row -> (pass, stream position)
        row_pass = np.zeros(nrows, np.int64)
        row_pos = np.full(nrows, -1, np.int64)
        idxs = [np.zeros(CA * 128, np.int64), np.zeros(CB * 128, np.int64)]
        taken = {}
        for sl in slots:
            kq = wid_base[sl["g"]] + _win_index(group_wins[sl["g"]], sl["woff"])
            if kq not in wstart:
                continue
            p = sl["p"]
            tA, tB = taken.get(kq, (0, 0))
            na = nA[kq]
            if p == 0:
                n = min(128, max(0, na - tA))
                b0 = wstart[kq] + tA
            else:
                n = min(128, max(0, wcount[kq] - na - tB))
                b0 = wstart[kq] + na + tB
            if not n:
                continue
            rows = order[b0:b0 + n]
            ss = r_src[rows] - (OVL if p else 0)
            assert ss.min() >= 0 and ss.max() < SPLIT
            idxs[p][sl["chunk"] * 128:sl["chunk"] * 128 + n] = ss
            row_pass[rows] = p
            row_pos[rows] = sl["scol"] * 0 + (sl["chunk"] * 128 + np.arange(n))
            # remember the S column base for rows of this slot via chunk->scol
            taken[kq] = (tA + n, tB) if p == 0 else (tA, tB + n)
        # S: per edge, its row's slot scol + in-window dest offset
        chunk_scol = [np.zeros(CA, np.int64), np.zeros(CB, np.int64)]
        for sl in slots:
            chunk_scol[sl["p"]][sl["chunk"]] = sl["scol"]
        assert (row_pos >= 0).all()
        e_rows = inv
        e_pass = row_pass[e_rows]
        e_chunk = row_pos[e_rows] // 128
        e_prow = row_pos[e_rows] % 128
        e_scol = np.where(e_pass == 0, chunk_scol[0][np.clip(e_chunk, 0, CA - 1)],
                          chunk_scol[1][np.clip(e_chunk, 0, CB - 1)])
        Sf = np.zeros((128, SCOLS), np.float32)
        np.add.at(Sf, (e_prow, e_scol + e_off), 1.0)
        per_core.append({
            "idxA": idxs[0].astype(np.int16),
            "idxB": idxs[1].astype(np.int16),
            "S": Sf.astype(ml_dtypes.float8_e4m3fn),
        })
    return slots, CA, CB, SCOLS, per_core


def _win_index(wins, woff):
    for i, (o, _, _, _) in enumerate(wins):
        if o == woff:
            return i
    raise KeyError(woff)


def _wrap_idx(idx):
    """dma_gather index layout: element i at [i % 16, i // 16], tiled to 128
    partitions."""
    w = np.ascontiguousarray(idx.reshape(-1, 16).T)  # [16, L/16]
    return np.tile(w, (8, 1))


def _make_batches(total, ramp=(8, 16, 32), cap=GCAP, tailmax=16):
    """Batch sizes: small ramp-in, cap-sized middle, small final batch."""
    sizes = []
    left = total
    for r in ramp:
        if left <= 0:
            break
        s = min(r, left)
        sizes.append(s)
        left -= s
    while left > 0:
        s = min(cap, left)
        sizes.append(s)
        left -= s
    if len(sizes) > 1 and sizes[-1] > tailmax:
        sizes[-1] -= tailmax
        sizes.append(tailmax)
    out = []
    st = 0
    for s in sizes:
        out.append((st, s))
        st += s
    return out


def _make_sbatches(slots, first=1024, cap=SCAP):
    """S-stream batches cut at slot boundaries: list of (startcol, ncols)."""
    bounds = [0]
    lim = first
    for sl in slots:
        end = sl["scol"] + sl["width"]
        if end - bounds[-1] > lim:
            bounds.append(sl["scol"])
            lim = cap
    total = slots[-1]["scol"] + slots[-1]["width"]
    bounds.append(total)
    return [(bounds[i], bounds[i + 1] - bounds[i]) for i in range(len(bounds) - 1)
            if bounds[i + 1] > bounds[i]]


def _build_program(slots, CA, CB, SCOLS):
    import concourse.bacc as bacc
    import concourse.tile as tile
    import concourse.mybir as mybir

    F32 = mybir.dt.float32
    F16 = mybir.dt.float16
    FP8 = mybir.dt.float8e4
    I16 = mybir.dt.int16

    nc = bacc.Bacc("TRN2", num_swdge_queues=4, dynamic_dma_scratch_size=98304)
    xa_d = nc.dram_tensor("xa", [N, H], F16, kind="ExternalInput")
    xaT_d = nc.dram_tensor("xaT", [H, NCOL], F16, kind="ExternalInput")
    idx_d = nc.dram_tensor("idx", [128, (CA + CB) * 8], I16, kind="ExternalInput")
    s_d = nc.dram_tensor("smat", [128, SCOLS], FP8, kind="ExternalInput")
    wagg_d = nc.dram_tensor("wagg", [H, H], F16, kind="ExternalInput")
    wx_d = nc.dram_tensor("wx", [H, H], F16, kind="ExternalInput")
    wo_d = nc.dram_tensor("wo", [H, H], F16, kind="ExternalInput")
    bh_d = nc.dram_tensor("bh", [H, 1], F32, kind="ExternalInput")
    outT_d = nc.dram_tensor("outT", [H, NCOL], F16, kind="ExternalOutput")

    # per-group chunk ranges (processing order) per pass, for tail alignment
    granges = [[], []]
    seen = []
    for sl in slots:
        if sl["pg"] not in seen:
            seen.append(sl["pg"])
            granges[0].append([None, None])
            granges[1].append([None, None])
        r = granges[sl["p"]][-1]
        c = sl["chunk"]
        r[0] = c if r[0] is None else min(r[0], c)
        r[1] = c + 1 if r[1] is None else max(r[1], c + 1)
    TAILG = 10

    def mk(total, ranges):
        tail_ranges = [r for r in ranges[-TAILG:] if r[0] is not None]
        head_end = min((r[0] for r in tail_ranges), default=total)
        sizes = []
        left = head_end
        for r in (8, 16, 32):
            if left <= 0:
                break
            t = min(r, left)
            sizes.append(t)
            left -= t
        while left > 0:
            t = min(GCAP, left)
            sizes.append(t)
            left -= t
        out = []
        st = 0
        for t in sizes:
            out.append((st, t))
            st += t
        for r in tail_ranges:
            if r[1] > st:
                out.append((st, r[1] - st))
                st = r[1]
        assert st == total, (st, total)
        return out

    batches = [mk(CA, granges[0]), mk(CB, granges[1])]
    gbmax = max(n for bl in batches for _, n in bl)
    sbatches = _make_sbatches(slots)
    scmax = max(n for _, n in sbatches)
    base = [(0, SPLIT), (OVL, N)]
    idx_off = [0, CA * 8]   # column offset of each pass in the idx tile

    relu = mybir.ActivationFunctionType.Relu
    copyf = mybir.ActivationFunctionType.Copy

    with tile.TileContext(nc) as tc:
        with (
            tc.tile_pool(name="const", bufs=1) as constp,
            tc.tile_pool(name="gath", bufs=3) as gathp,
            tc.tile_pool(name="spool", bufs=3) as spool,
            tc.tile_pool(name="post", bufs=2) as postp,
            tc.tile_pool(name="ps", bufs=2, space="PSUM") as psump,
            tc.tile_pool(name="psa", bufs=3, space="PSUM") as psumpa,
        ):
            # whole idx stream resident; head slice first so batch 0 starts fast
            idx_t = constp.tile([128, (CA + CB) * 8], I16, tag="idx")
            headA = min(CA, batches[0][0][1])
            headB = min(CB, batches[1][0][1])
            nc.sync.dma_start(idx_t[:, :headA * 8], idx_d[:, :headA * 8])
            nc.sync.dma_start(idx_t[:, CA * 8:CA * 8 + headB * 8],
                              idx_d[:, CA * 8:CA * 8 + headB * 8])
            if CA > headA:
                nc.sync.dma_start(idx_t[:, headA * 8:CA * 8],
                                  idx_d[:, headA * 8:CA * 8])
            if CB > headB:
                nc.sync.dma_start(idx_t[:, CA * 8 + headB * 8:],
                                  idx_d[:, CA * 8 + headB * 8:])

            wagg_t = constp.tile([H, H], F16, tag="wagg")
            wx_t = constp.tile([H, H], F16, tag="wx")
            wo_t = constp.tile([H, H], F16, tag="wo")
            bh_t = constp.tile([H, 1], F32, tag="bh")
            xaT_t = constp.tile([128, NCOL], F16, tag="xaT")
            for t, dd in ((wagg_t, wagg_d), (wx_t, wx_d), (wo_t, wo_d),
                          (bh_t, bh_d), (xaT_t, xaT_d)):
                nc.sync.dma_start(t[:], dd[:])

            # streaming state
            cur_g = [None, None]      # current gather tile per pass
            gstart = [0, 0]
            gpos = [0, 0]             # next batch index per pass
            consumed = [0, 0]
            cur_s = [None]            # current S tile
            sstart = [0]
            spos = [0]
            qrr = [0]

            def lhsT_for(p, c):
                if cur_g[p] is None or c >= gstart[p] + cur_g[p].shape[1]:
                    st, nch = batches[p][gpos[p]]
                    assert st == c, (p, c, st)
                    gpos[p] += 1
                    gstart[p] = st
                    t = gathp.tile([128, gbmax, H], F16, tag=f"g{p}")
                    t = t[:, :nch, :]
                    lo, hi = base[p]
                    nc.gpsimd.dma_gather(
                        t[:], xa_d[lo:hi, :],
                        idx_t[:, idx_off[p] + st * 8: idx_off[p] + (st + nch) * 8],
                        nch * 128, nch * 128, H,
                        single_packet=False, queue_num=qrr[0] % 4,
                    )
                    qrr[0] += 1
                    cur_g[p] = t
                return cur_g[p][:, c - gstart[p], :]

            def s_for(scol, w):
                if cur_s[0] is None or scol >= sstart[0] + cur_s[0].shape[1]:
                    st, ncols = sbatches[spos[0]]
                    assert st == scol, (scol, st)
                    spos[0] += 1
                    sstart[0] = st
                    t = spool.tile([128, scmax], FP8, tag="s")
                    t = t[:, :ncols]
                    nc.sync.dma_start(t[:], s_d[:, st:st + ncols])
                    cur_s[0] = t
                return cur_s[0][:, scol - sstart[0]: scol - sstart[0] + w]

            aggr_ps = None
            for sl in slots:
                if sl["first"]:
                    aggr_ps = psumpa.tile([128, GROUP], F32, tag="aggr")
                p = sl["p"]
                lhsT = lhsT_for(p, consumed[p])
                consumed[p] += 1
                rhs = s_for(sl["scol"], sl["width"])
                nc.tensor.matmul(
                    aggr_ps[:, sl["woff"]:sl["woff"] + sl["width"]],
                    lhsT, rhs, start=sl["first"], stop=sl["last"],
                )
                if sl["last"]:
                    g = sl["g"]
                    c0, c1 = sl["rng"]
                    w = c1 - c0
                    aggr_sb = postp.tile([128, GROUP], F16, tag="aggr_sb")
                    z_ps = psump.tile([128, GROUP], F32, tag="z")
                    h_sb = postp.tile([128, GROUP], F16, tag="h")
                    o_ps = psump.tile([128, GROUP], F32, tag="o")
                    o_sb = postp.tile([128, GROUP], F16, tag="osb")
                    nc.vector.tensor_scalar_mul(aggr_sb[:, :w], aggr_ps[:, c0:c1], 1.0)
                    nc.tensor.matmul(z_ps[:, :w], wagg_t[:], aggr_sb[:, :w],
                                     start=True, stop=False)
                    nc.tensor.matmul(z_ps[:, :w], wx_t[:],
                                     xaT_t[:, g * GROUP + c0:g * GROUP + c1],
                                     start=False, stop=True)
                    nc.vector.tensor_scalar(
                        out=h_sb[:, :w], in0=z_ps[:, :w],
                        scalar1=bh_t[:, 0:1], scalar2=0.0,
                        op0=mybir.AluOpType.add, op1=mybir.AluOpType.max)
                    nc.tensor.matmul(o_ps[:, :w], wo_t[:], h_sb[:, :w],
                                     start=True, stop=True)
                    nc.scalar.activation(o_sb[:, :w], o_ps[:, :w], copyf)
                    nc.sync.dma_start(outT_d[:, g * GROUP + c0:g * GROUP + c1],
                                      o_sb[:, :w])

    nc.compile()
    return nc


def prepare(inputs):
    """Host-side packing: returns (nc, in_maps)."""
    x_a = np.ascontiguousarray(np.asarray(inputs["x_a"], dtype=np.float32))
    eb = np.asarray(inputs["edge_ba"])
    dst = eb[0].astype(np.int64)
    src = eb[1].astype(np.int64)

    wagg = np.ascontiguousarray(
        np.asarray(inputs["conv1_wl_w"], np.float32).T.astype(np.float16))
    wx = np.ascontiguousarray(
        (np.asarray(inputs["conv1_w0_w"], np.float32)
         + np.asarray(inputs["conv1_w1_w"], np.float32)).T.astype(np.float16))
    bh = (np.asarray(inputs["conv1_wl_b"], np.float32)
          + np.asarray(inputs["conv1_w0_b"], np.float32)
          + np.asarray(inputs["conv1_w1_b"], np.float32)).reshape(H, 1)
    wo = np.ascontiguousarray(np.asarray(inputs["out_w"], np.float32).T.astype(np.float16))
    xa16 = x_a.astype(np.float16)

    # degree-balanced dest sharding: deal degree-sorted dests round-robin so
    # every core's window k sees near-identical load (shared chunk budgets
    # are maxed over cores -- this kills the max-over-cores padding)
    deg = np.bincount(dst, minlength=N)
    dorder = np.argsort(-deg, kind="stable")
    core_of = np.empty(N, np.int64)
    dl_of = np.empty(N, np.int64)
    ar = np.arange(N)
    core_of[dorder] = ar % P
    # stratified: spread the degree spectrum across groups so group weights
    # are uniform (only one group's post chain lands in the tail)
    r2 = ar // P
    dl_of[dorder] = (r2 % NG) * GROUP + (r2 // NG)

    slots, CA, CB, SCOLS, per_core = _pack_edges(dst, src, core_of, dl_of)
    nc = _build_program(slots, CA, CB, SCOLS)

    r2s = np.arange(NSH)
    cols = (r2s % NG) * GROUP + (r2s // NG)
    in_maps = []
    for c in range(P):
        xaT = np.zeros((H, NCOL), np.float16)
        xaT[:, cols] = x_a[dorder[r2s * P + c]].T.astype(np.float16)
        a = per_core[c]
        idx = np.concatenate([
            _wrap_idx(a["idxA"]), _wrap_idx(a["idxB"])], axis=1)
        in_maps.append({
            "xa": xa16,
            "xaT": xaT,
            "idx": idx,
            "smat": a["S"],
            "wagg": wagg, "wx": wx, "wo": wo, "bh": bh,
        })
    return nc, in_maps, np.asarray(inputs["out_b"], np.float32), dorder


def assemble(results, out_b, dorder):
    out = np.empty((N, H), np.float32)
    r2s = np.arange(NSH)
    cols = (r2s % NG) * GROUP + (r2s // NG)
    for c in range(P):
        out[dorder[r2s * P + c]] = results[c]["outT"][:, cols].T.astype(np.float32)
    out += out_b[None, :]
    return out


def kernel(**inputs):
    from concourse.bass_utils import run_bass_kernel_spmd

    nc, in_maps, out_b, dorder = prepare(inputs)
    r = run_bass_kernel_spmd(nc, in_maps, list(range(P)))
    return assemble(r.results, out_b, dorder)


# revision 37
# speedup vs baseline: 1.0294x; 1.0294x over previous
"""MetaPathGNN kernel for 8 Trainium2 NeuronCores.

Computation (h_b/conv0/edge_ab/x_b are dead code in the reference):
    msg  = x_a[edge_ba[1]]                      # [E, H] gather
    aggr = segment_sum(msg, edge_ba[0], N)      # [N, H]
    h_a  = relu(aggr @ wl1.T + x_a @ (w01+w11).T + (bl1+b01+b11))
    out  = h_a @ out_w.T + out_b

Sharding: destination nodes are dealt to the 8 cores round-robin in
descending-degree order (so the shared SPMD chunk budgets, which are maxed
across cores, see near-identical per-window loads), stratified across PSUM
groups so group weights are uniform. Each core gathers source rows for its
own edges from a full x_a replica (no collectives), aggregates via one-hot
matmuls into PSUM, applies the linears feature-major, writes its outT
stripe; the host un-permutes and reassembles.

Cost-model structure this kernel is built around (measured):
  - SWDGE gather transfers serialize at ~0.833 ns/row (256B f16 rows) and are
    the wall; everything else must hide under them.
  - HWDGE (regular dma_start) traffic runs fully in parallel with the SWDGE
    stream (~330 GB/s), so the one-hot S matrices are PRECOMPUTED ON HOST,
    stored fp8 (exact for 0/1), and streamed from HBM instead of being built
    on DVE (matmul accepts mixed f16 lhsT x fp8 rhs).
  - Each dma_start costs ~0.5-0.8us of SP engine time regardless of size, so
    loads are batched coarsely (whole idx stream, whole xaT, few S batches).

Aggregation: per 250-dest PSUM group (25 groups), windows of <=128 dests are
chosen by a small DP to minimize the shared chunk budget sum_w max_core
ceil(rows/128) on source-DEDUPED row counts (a gathered row's S row is
multi-hot over all its edges in the window; duplicate edges add 2). Pad
chunks gather row 0 with an all-zero S block. Window matmuls out = msg.T @ S
accumulate bank-wide (start=True pending-zeroes the bank; every window is
touched by >=1 matmul). The last TAILG groups get group-aligned gather
batches so their post chains pipeline with the stream instead of piling up
after it.

dma_gather indices are int16, so sources are gathered in two passes with
OVERLAPPING bases (pass A: rows [0, 32768), pass B: rows [17232, 50000)) --
edges with src in the overlap can ride in either pass, which merges the
per-window per-pass ceil waste in the shared chunk budgets.
"""

import numpy as np

P = 8
N = 50000
E = 500000
H = 128
NSH = N // P          # 6250 destinations per core
GROUP = 256           # aggregation group width (<= 512 psum bank cols)
NG = (NSH + GROUP - 1) // GROUP   # 25
NRG = NSH // NG       # 250 real dests per group (stratified deal)
NCOL = NG * GROUP
SPLIT = 32768         # int16-index limit for dma_gather
OVL = N - SPLIT       # 17232: pass-B gather base; src in [OVL, SPLIT) fits either pass
WMAX = 128            # max window width
GCAP = 48             # gather batch cap, chunks
SCAP = 4096           # S-stream batch cap, columns


def _bucket_budget(cc, s, e):
    """Shared (bA, bB) chunk budget for window [s, e).

    cc: [P, 3, n+1] cumsums by class (0=must-be-A src<OVL, 1=flexible,
    2=must-be-B src>=SPLIT). Flexible edges may go in either pass, so the
    total budget is max(ceil(total), ceil(minA)+ceil(minB)) per core-max.
    """
    nAm = int((cc[:, 0, e] - cc[:, 0, s]).max())
    nBm = int((cc[:, 2, e] - cc[:, 2, s]).max())
    nT = int(((cc[:, 0, e] + cc[:, 1, e] + cc[:, 2, e])
              - (cc[:, 0, s] + cc[:, 1, s] + cc[:, 2, s])).max())
    LA = -(-nAm // 128)
    LB = -(-nBm // 128)
    bt = max(-(-nT // 128), LA + LB, 1)
    bA = max(LA, bt - LB)
    return bA, bt - bA


def _dp_windows(cc, nreal):
    """Choose window boundaries for one group: tile [0, nreal) with windows
    <= WMAX wide, minimizing total chunk budget (ties: fewer matmul cols)."""
    INF = 1 << 40
    dp = np.full(nreal + 1, INF, np.int64)
    dp[0] = 0
    choice = np.zeros(nreal + 1, np.int64)
    ccT = cc.sum(axis=1)
    for e in range(1, nreal + 1):
        w = np.arange(1, min(WMAX, e) + 1)
        s = e - w
        nAm = (cc[:, 0, e:e + 1] - cc[:, 0, s]).max(axis=0)
        nBm = (cc[:, 2, e:e + 1] - cc[:, 2, s]).max(axis=0)
        nT = (ccT[:, e:e + 1] - ccT[:, s]).max(axis=0)
        bt = np.maximum(np.maximum(-(-nT // 128), (-(-nAm // 128)) + (-(-nBm // 128))), 1)
        cost = dp[s] + bt * 100000 + bt * w
        i = int(np.argmin(cost))
        dp[e] = cost[i]
        choice[e] = w[i]
    wins = []
    e = nreal
    while e > 0:
        w = int(choice[e])
        s = e - w
        bA, bB = _bucket_budget(cc, s, e)
        wins.append((s, w, bA, bB))
        e = s
    return wins[::-1]


def _pack_edges(dst, src, core_of, dl_of):
    """Window selection, shared slot schedule, and per-core stream packing.

    Returns (slots, CA, CB, SCOLS, per_core).
    slots: list of dicts with p, g, woff, width, scol, chunk (per-pass chunk
    index), first/last (of PSUM group) in processing order.
    per_core[c]: idxA/idxB int16 streams and S fp8 [128, SCOLS].
    """
    core = core_of[dst]
    dl = dl_of[dst]
    g_of = dl // GROUP
    lo = dl - g_of * GROUP
    cls = np.where(src < OVL, 0, np.where(src < SPLIT, 1, 2)).astype(np.int64)

    # per-group per-core per-class cumulative counts -> DP windows
    group_wins = []
    for g in range(NG):
        nreal = NRG
        cc = np.zeros((P, 3, nreal + 1), np.int64)
        m = g_of == g
        for c in range(P):
            for k in range(3):
                mm = m & (core == c) & (cls == k)
                cnt = np.bincount(lo[mm], minlength=nreal)
                cc[c, k, 1:] = np.cumsum(cnt)
        group_wins.append(_dp_windows(cc, nreal))

    # process heavy groups first so the tail chain is light
    gweight = [sum(bA + bB for _, _, bA, bB in group_wins[g]) for g in range(NG)]
    group_order = sorted(range(NG), key=lambda g: -gweight[g])

    slots = []
    ca = cb = scols = 0
    for g in group_order:
        first = len(slots)
        for woff, w, bA, bB in group_wins[g]:
            for p, b in ((0, bA), (1, bB)):
                for _ in range(b):
                    slots.append(dict(p=p, g=g, woff=woff, width=w, scol=scols,
                                      chunk=(ca if p == 0 else cb),
                                      first=False, last=False))
                    scols += w
                    if p == 0:
                        ca += 1
                    else:
                        cb += 1
        slots[first]["first"] = True
        slots[-1]["last"] = True
    CA, CB, SCOLS = ca, cb, scols

    # per-core packing: bucket edges by (g, window, pass), chunk, emit streams
    import ml_dtypes
    win_id = np.zeros(len(dl), np.int64)
    win_off = np.zeros(len(dl), np.int64)
    wid_base = {}
    wb = 0
    for g in range(NG):
        bounds = np.array([woff for woff, _, _, _ in group_wins[g]] + [GROUP])  # edges all < nreal
        m = g_of == g
        wi = np.searchsorted(bounds, lo[m], side="right") - 1
        win_id[m] = wb + wi
        win_off[m] = lo[m] - bounds[wi]
        wid_base[g] = wb
        wb += len(group_wins[g])

    # ---- row-level (source-deduped) packing ----
    # A gathered row is (window, source); its S row is multi-hot over the
    # window columns of all its edges (duplicate (src,dst) edges add 2).
    # Budgets are therefore computed on DISTINCT-source counts per window.
    NWID = wb
    row_data = []   # per core: (row_win, row_src, row_cls, edge_inv, edge_scolpos)
    cnts = np.zeros((P, NWID, 3), np.int64)
    for c in range(P):
        m = core == c
        w_ = win_id[m]
        s_ = src[m]
        pair = w_ * N + s_
        uniq, inv = np.unique(pair, return_inverse=True)
        r_win = uniq // N
        r_src = uniq % N
        r_cls = np.where(r_src < OVL, 0, np.where(r_src < SPLIT, 1, 2))
        np.add.at(cnts[c], (r_win, r_cls), 1)
        row_data.append((r_win, r_src, r_cls, inv, win_off[m]))

    # shared budgets per window from distinct counts (flex merges the ceils)
    def _budget(wid):
        nAm = int(cnts[:, wid, 0].max())
        nBm = int(cnts[:, wid, 2].max())
        nT = int(cnts[:, wid, :].sum(axis=1).max())
        LA = -(-nAm // 128)
        LB = -(-nBm // 128)
        bt = max(-(-nT // 128), LA + LB, 1)
        bA = max(LA, bt - LB)
        return bA, bt - bA

    new_wins = []
    for g in range(NG):
        nw = []
        for i, (woff, w, _, _) in enumerate(group_wins[g]):
            bA, bB = _budget(wid_base[g] + i)
            nw.append((woff, w, bA, bB))
        new_wins.append(nw)
    group_wins = new_wins

    # rebuild slots with the tightened budgets
    gweight = [sum(bA + bB for _, _, bA, bB in group_wins[g]) for g in range(NG)]
    group_order = sorted(range(NG), key=lambda g: -gweight[g])
    slots = []
    ca = cb = scols = 0
    pg = 0
    for gi, g in enumerate(group_order):
        wins = group_wins[g]
        if gi == len(group_order) - 1 and len(wins) > 1:
            # final group: two pseudo-groups (separate PSUM banks + stop
            # flags + batches) so its first half-chain starts ~1us before
            # the gather stream ends
            k = min(range(1, len(wins)),
                    key=lambda i: abs(wins[i][0] - NRG // 2))
            subs = [(wins[:k], (0, wins[k][0])), (wins[k:], (wins[k][0], NRG))]
        else:
            subs = [(wins, (0, NRG))]
        for sub, rng in subs:
            first = len(slots)
            for woff, w, bA, bB in sub:
                for p, b in ((0, bA), (1, bB)):
                    for _ in range(b):
                        slots.append(dict(p=p, g=g, pg=pg, woff=woff, width=w,
                                          scol=scols,
                                          chunk=(ca if p == 0 else cb),
                                          first=False, last=False, rng=rng))
                        scols += w
                        if p == 0:
                            ca += 1
                        else:
                            cb += 1
            slots[first]["first"] = True
            slots[-1]["last"] = True
            pg += 1
    CA, CB, SCOLS = ca, cb, scols

    budA = {}
    for g in range(NG):
        for i, (woff, w, bA, bB) in enumerate(group_wins[g]):
            budA[wid_base[g] + i] = bA

    per_core = []
    for c in range(P):
        r_win, r_src, r_cls, inv, e_off = row_data[c]
        nrows = len(r_win)
        # rows ordered by (window, class, src); rank within window
        order = np.lexsort((r_src, r_cls, r_win))
        rank = np.empty(nrows, np.int64)
        rank[order] = np.arange(nrows)
        uw, w_first = np.unique(r_win[order], return_index=True)
        wstart = {int(u): int(fi) for u, fi in zip(uw, w_first)}
        wcount = {int(u): int(n) for u, n in
                  zip(uw, np.bincount(np.searchsorted(uw, r_win[order])))}
        # A-row count per window: must-A + flex up to the A budget
        nA = {}
        for u in uw:
            u = int(u)
            n_mustA = int(cnts[c, u, 0])
            n_flex = int(cnts[c, u, 1])
            nA[u] = min(n_mustA + n_flex, 128 * budA[u])
        # map each row -> (pass, stream position)
        row_pass = np.zeros(nrows, np.int64)
        row_pos = np.full(nrows, -1, np.int64)
        idxs = [np.zeros(CA * 128, np.int64), np.zeros(CB * 128, np.int64)]
        taken = {}
        for sl in slots:
            kq = wid_base[sl["g"]] + _win_index(group_wins[sl["g"]], sl["woff"])
            if kq not in wstart:
                continue
            p = sl["p"]
            tA, tB = taken.get(kq, (0, 0))
            na = nA[kq]
            if p == 0:
                n = min(128, max(0, na - tA))
                b0 = wstart[kq] + tA
            else:
                n = min(128, max(0, wcount[kq] - na - tB))
                b0 = wstart[kq] + na + tB
            if not n:
                continue
            rows = order[b0:b0 + n]
            ss = r_src[rows] - (OVL if p else 0)
            assert ss.min() >= 0 and ss.max() < SPLIT
            idxs[p][sl["chunk"] * 128:sl["chunk"] * 128 + n] = ss
            row_pass[rows] = p
            row_pos[rows] = sl["scol"] * 0 + (sl["chunk"] * 128 + np.arange(n))
            # remember the S column base for rows of this slot via chunk->scol
            taken[kq] = (tA + n, tB) if p == 0 else (tA, tB + n)
        # S: per edge, its row's slot scol + in-window dest offset
        chunk_scol = [np.zeros(CA, np.int64), np.zeros(CB, np.int64)]
        for sl in slots:
            chunk_scol[sl["p"]][sl["chunk"]] = sl["scol"]
        assert (row_pos >= 0).all()
        e_rows = inv
        e_pass = row_pass[e_rows]
        e_chunk = row_pos[e_rows] // 128
        e_prow = row_pos[e_rows] % 128
        e_scol = np.where(e_pass == 0, chunk_scol[0][np.clip(e_chunk, 0, CA - 1)],
                          chunk_scol[1][np.clip(e_chunk, 0, CB - 1)])
        Sf = np.zeros((128, SCOLS), np.float32)
        np.add.at(Sf, (e_prow, e_scol + e_off), 1.0)
        per_core.append({
            "idxA": idxs[0].astype(np.int16),
            "idxB": idxs[1].astype(np.int16),
            "S": Sf.astype(ml_dtypes.float8_e4m3fn),
        })
    return slots, CA, CB, SCOLS, per_core


def _win_index(wins, woff):
    for i, (o, _, _, _) in enumerate(wins):
        if o == woff:
            return i
    raise KeyError(woff)


def _wrap_idx(idx):
    """dma_gather index layout: element i at [i % 16, i // 16], tiled to 128
    partitions."""
    w = np.ascontiguousarray(idx.reshape(-1, 16).T)  # [16, L/16]
    return np.tile(w, (8, 1))


def _make_batches(total, ramp=(8, 16, 32), cap=GCAP, tailmax=16):
    """Batch sizes: small ramp-in, cap-sized middle, small final batch."""
    sizes = []
    left = total
    for r in ramp:
        if left <= 0:
            break
        s = min(r, left)
        sizes.append(s)
        left -= s
    while left > 0:
        s = min(cap, left)
        sizes.append(s)
        left -= s
    if len(sizes) > 1 and sizes[-1] > tailmax:
        sizes[-1] -= tailmax
        sizes.append(tailmax)
    out = []
    st = 0
    for s in sizes:
        out.append((st, s))
        st += s
    return out


def _make_sbatches(slots, first=1024, cap=SCAP):
    """S-stream batches cut at slot boundaries: list of (startcol, ncols)."""
    bounds = [0]
    lim = first
    for sl in slots:
        end = sl["scol"] + sl["width"]
        if end - bounds[-1] > lim:
            bounds.append(sl["scol"])
            lim = cap
    total = slots[-1]["scol"] + slots[-1]["width"]
    bounds.append(total)
    return [(bounds[i], bounds[i + 1] - bounds[i]) for i in range(len(bounds) - 1)
            if bounds[i + 1] > bounds[i]]


def _build_program(slots, CA, CB, SCOLS):
    import concourse.bacc as bacc
    import concourse.tile as tile
    import concourse.mybir as mybir

    F32 = mybir.dt.float32
    F16 = mybir.dt.float16
    FP8 = mybir.dt.float8e4
    I16 = mybir.dt.int16

    nc = bacc.Bacc("TRN2", num_swdge_queues=4, dynamic_dma_scratch_size=98304)
    xa_d = nc.dram_tensor("xa", [N, H], F16, kind="ExternalInput")
    xaT_d = nc.dram_tensor("xaT", [H, NCOL], F16, kind="ExternalInput")
    idx_d = nc.dram_tensor("idx", [128, (CA + CB) * 8], I16, kind="ExternalInput")
    s_d = nc.dram_tensor("smat", [128, SCOLS], FP8, kind="ExternalInput")
    wagg_d = nc.dram_tensor("wagg", [H, H], F16, kind="ExternalInput")
    wx_d = nc.dram_tensor("wx", [H, H], F16, kind="ExternalInput")
    wo_d = nc.dram_tensor("wo", [H, H], F16, kind="ExternalInput")
    bh_d = nc.dram_tensor("bh", [H, 1], F32, kind="ExternalInput")
    outT_d = nc.dram_tensor("outT", [H, NCOL], F16, kind="ExternalOutput")

    # per-group chunk ranges (processing order) per pass, for tail alignment
    granges = [[], []]
    seen = []
    for sl in slots:
        if sl["pg"] not in seen:
            seen.append(sl["pg"])
            granges[0].append([None, None])
            granges[1].append([None, None])
        r = granges[sl["p"]][-1]
        c = sl["chunk"]
        r[0] = c if r[0] is None else min(r[0], c)
        r[1] = c + 1 if r[1] is None else max(r[1], c + 1)
    TAILG = 10

    def mk(total, ranges):
        tail_ranges = [r for r in ranges[-TAILG:] if r[0] is not None]
        head_end = min((r[0] for r in tail_ranges), default=total)
        sizes = []
        left = head_end
        for r in (8, 16, 32):
            if left <= 0:
                break
            t = min(r, left)
            sizes.append(t)
            left -= t
        while left > 0:
            t = min(GCAP, left)
            sizes.append(t)
            left -= t
        out = []
        st = 0
        for t in sizes:
            out.append((st, t))
            st += t
        for r in tail_ranges:
            if r[1] > st:
                out.append((st, r[1] - st))
                st = r[1]
        assert st == total, (st, total)
        return out

    batches = [mk(CA, granges[0]), mk(CB, granges[1])]
    gbmax = max(n for bl in batches for _, n in bl)
    sbatches = _make_sbatches(slots)
    scmax = max(n for _, n in sbatches)
    base = [(0, SPLIT), (OVL, N)]
    idx_off = [0, CA * 8]   # column offset of each pass in the idx tile

    relu = mybir.ActivationFunctionType.Relu
    copyf = mybir.ActivationFunctionType.Copy

    with tile.TileContext(nc) as tc:
        with (
            tc.tile_pool(name="const", bufs=1) as constp,
            tc.tile_pool(name="gath", bufs=3) as gathp,
            tc.tile_pool(name="spool", bufs=3) as spool,
            tc.tile_pool(name="post", bufs=2) as postp,
            tc.tile_pool(name="ps", bufs=2, space="PSUM") as psump,
            tc.tile_pool(name="psa", bufs=3, space="PSUM") as psumpa,
        ):
            # whole idx stream resident; head slice first so batch 0 starts fast
            idx_t = constp.tile([128, (CA + CB) * 8], I16, tag="idx")
            headA = min(CA, batches[0][0][1])
            headB = min(CB, batches[1][0][1])
            nc.sync.dma_start(idx_t[:, :headA * 8], idx_d[:, :headA * 8])
            nc.sync.dma_start(idx_t[:, CA * 8:CA * 8 + headB * 8],
                              idx_d[:, CA * 8:CA * 8 + headB * 8])
            if CA > headA:
                nc.sync.dma_start(idx_t[:, headA * 8:CA * 8],
                                  idx_d[:, headA * 8:CA * 8])
            if CB > headB:
                nc.sync.dma_start(idx_t[:, CA * 8 + headB * 8:],
                                  idx_d[:, CA * 8 + headB * 8:])

            wagg_t = constp.tile([H, H], F16, tag="wagg")
            wx_t = constp.tile([H, H], F16, tag="wx")
            wo_t = constp.tile([H, H], F16, tag="wo")
            bh_t = constp.tile([H, 1], F32, tag="bh")
            xaT_t = constp.tile([128, NCOL], F16, tag="xaT")
            for t, dd in ((wagg_t, wagg_d), (wx_t, wx_d), (wo_t, wo_d),
                          (bh_t, bh_d), (xaT_t, xaT_d)):
                nc.sync.dma_start(t[:], dd[:])

            # streaming state
            cur_g = [None, None]      # current gather tile per pass
            gstart = [0, 0]
            gpos = [0, 0]             # next batch index per pass
            consumed = [0, 0]
            cur_s = [None]            # current S tile
            sstart = [0]
            spos = [0]
            qrr = [0]

            def lhsT_for(p, c):
                if cur_g[p] is None or c >= gstart[p] + cur_g[p].shape[1]:
                    st, nch = batches[p][gpos[p]]
                    assert st == c, (p, c, st)
                    gpos[p] += 1
                    gstart[p] = st
                    t = gathp.tile([128, gbmax, H], F16, tag=f"g{p}")
                    t = t[:, :nch, :]
                    lo, hi = base[p]
                    nc.gpsimd.dma_gather(
                        t[:], xa_d[lo:hi, :],
                        idx_t[:, idx_off[p] + st * 8: idx_off[p] + (st + nch) * 8],
                        nch * 128, nch * 128, H,
                        single_packet=False, queue_num=qrr[0] % 4,
                    )
                    qrr[0] += 1
                    cur_g[p] = t
                return cur_g[p][:, c - gstart[p], :]

            def s_for(scol, w):
                if cur_s[0] is None or scol >= sstart[0] + cur_s[0].shape[1]:
                    st, ncols = sbatches[spos[0]]
                    assert st == scol, (scol, st)
                    spos[0] += 1
                    sstart[0] = st
                    t = spool.tile([128, scmax], FP8, tag="s")
                    t = t[:, :ncols]
                    nc.sync.dma_start(t[:], s_d[:, st:st + ncols])
                    cur_s[0] = t
                return cur_s[0][:, scol - sstart[0]: scol - sstart[0] + w]

            aggr_ps = None
            for sl in slots:
                if sl["first"]:
                    aggr_ps = psumpa.tile([128, GROUP], F32, tag="aggr")
                p = sl["p"]
                lhsT = lhsT_for(p, consumed[p])
                consumed[p] += 1
                rhs = s_for(sl["scol"], sl["width"])
                nc.tensor.matmul(
                    aggr_ps[:, sl["woff"]:sl["woff"] + sl["width"]],
                    lhsT, rhs, start=sl["first"], stop=sl["last"],
                )
                if sl["last"]:
                    g = sl["g"]
                    c0, c1 = sl["rng"]
                    w = c1 - c0
                    aggr_sb = postp.tile([128, GROUP], F16, tag="aggr_sb")
                    z_ps = psump.tile([128, GROUP], F32, tag="z")
                    h_sb = postp.tile([128, GROUP], F16, tag="h")
                    o_ps = psump.tile([128, GROUP], F32, tag="o")
                    o_sb = postp.tile([128, GROUP], F16, tag="osb")
                    nc.vector.tensor_scalar_mul(aggr_sb[:, :w], aggr_ps[:, c0:c1], 1.0)
                    nc.tensor.matmul(z_ps[:, :w], wagg_t[:], aggr_sb[:, :w],
                                     start=True, stop=False)
                    nc.tensor.matmul(z_ps[:, :w], wx_t[:],
                                     xaT_t[:, g * GROUP + c0:g * GROUP + c1],
                                     start=False, stop=True)
                    nc.vector.tensor_scalar(
                        out=h_sb[:, :w], in0=z_ps[:, :w],
                        scalar1=bh_t[:, 0:1], scalar2=0.0,
                        op0=mybir.AluOpType.add, op1=mybir.AluOpType.max)
                    nc.tensor.matmul(o_ps[:, :w], wo_t[:], h_sb[:, :w],
                                     start=True, stop=True)
                    nc.scalar.activation(o_sb[:, :w], o_ps[:, :w], copyf)
                    nc.sync.dma_start(outT_d[:, g * GROUP + c0:g * GROUP + c1],
                                      o_sb[:, :w])

    nc.compile()
    return nc


def prepare(inputs):
    """Host-side packing: returns (nc, in_maps)."""
    x_a = np.ascontiguousarray(np.asarray(inputs["x_a"], dtype=np.float32))
    eb = np.asarray(inputs["edge_ba"])
    dst = eb[0].astype(np.int64)
    src = eb[1].astype(np.int64)

    wagg = np.ascontiguousarray(
        np.asarray(inputs["conv1_wl_w"], np.float32).T.astype(np.float16))
    wx = np.ascontiguousarray(
        (np.asarray(inputs["conv1_w0_w"], np.float32)
         + np.asarray(inputs["conv1_w1_w"], np.float32)).T.astype(np.float16))
    bh = (np.asarray(inputs["conv1_wl_b"], np.float32)
          + np.asarray(inputs["conv1_w0_b"], np.float32)
          + np.asarray(inputs["conv1_w1_b"], np.float32)).reshape(H, 1)
    wo = np.ascontiguousarray(np.asarray(inputs["out_w"], np.float32).T.astype(np.float16))
    xa16 = x_a.astype(np.float16)

    # degree-balanced dest sharding: deal degree-sorted dests round-robin so
    # every core's window k sees near-identical load (shared chunk budgets
    # are maxed over cores -- this kills the max-over-cores padding)
    deg = np.bincount(dst, minlength=N)
    dorder = np.argsort(-deg, kind="stable")
    core_of = np.empty(N, np.int64)
    dl_of = np.empty(N, np.int64)
    ar = np.arange(N)
    core_of[dorder] = ar % P
    # stratified: spread the degree spectrum across groups so group weights
    # are uniform (only one group's post chain lands in the tail)
    r2 = ar // P
    dl_of[dorder] = (r2 % NG) * GROUP + (r2 // NG)

    slots, CA, CB, SCOLS, per_core = _pack_edges(dst, src, core_of, dl_of)
    nc = _build_program(slots, CA, CB, SCOLS)

    r2s = np.arange(NSH)
    cols = (r2s % NG) * GROUP + (r2s // NG)
    in_maps = []
    for c in range(P):
        xaT = np.zeros((H, NCOL), np.float16)
        xaT[:, cols] = x_a[dorder[r2s * P + c]].T.astype(np.float16)
        a = per_core[c]
        idx = np.concatenate([
            _wrap_idx(a["idxA"]), _wrap_idx(a["idxB"])], axis=1)
        in_maps.append({
            "xa": xa16,
            "xaT": xaT,
            "idx": idx,
            "smat": a["S"],
            "wagg": wagg, "wx": wx, "wo": wo, "bh": bh,
        })
    return nc, in_maps, np.asarray(inputs["out_b"], np.float32), dorder


def assemble(results, out_b, dorder):
    out = np.empty((N, H), np.float32)
    r2s = np.arange(NSH)
    cols = (r2s % NG) * GROUP + (r2s // NG)
    for c in range(P):
        out[dorder[r2s * P + c]] = results[c]["outT"][:, cols].T.astype(np.float32)
    out += out_b[None, :]
    return out


def kernel(**inputs):
    from concourse.bass_utils import run_bass_kernel_spmd

    nc, in_maps, out_b, dorder = prepare(inputs)
    r = run_bass_kernel_spmd(nc, in_maps, list(range(P)))
    return assemble(r.results, out_b, dorder)
